# revision 37
# baseline (speedup 1.0000x reference)
"""Trainium2 Bass kernel for nn_CenterLossNet (center-loss softmax over classes).

Math (reference):
    f = l2_normalize(features); c = l2_normalize(centers)
    dis[n,k]  = -5 * (|f_n|^2 + |c_k|^2 - 2 f_n.c_k)        # [N, C]
    pos[n]    = dis[n, labels[n]] + bias[labels[n]]
    den[n]    = sum_k exp(dis[n,k]) - exp(dis[n,l_n]) + exp(pos[n])
    loss      = mean(log(den) - pos) + var(pos, ddof=1);  returns (loss, var)

Device does the heavy part: S = f_hat @ c_hat.T (8192x10000x512 matmul) fused
with exp(10*S + bias_n) on the scalar engine. The matmul runs in fp8e4m3
DoubleRow perf mode (operands pre-scaled by 2^9, two k-rows packed per PE
cell); the exp-sum averages the fp8 rounding noise down to ~1e-5 relative on
the loss. Row-sums of the bf16 exp tiles run on the vector engine
(tensor_scalar 4x mode with accum_out). Everything O(N) or O(C) runs on host
in fp64, so pos/variance use exact fp32 inputs.

Sharding: data-parallel over batch N across 8 cores; centers replicated.
For the row-sum the per-class |c_k|^2 term is folded as exactly 1.0 (the
normalized squared norms differ from 1 by ~1e-6, and the host applies the
mean residual correction), while pos[n] uses the exact fp32 per-label norms.
"""

import numpy as np
import ml_dtypes

import concourse.bacc as bacc
import concourse.mybir as mybir
import concourse.tile as tile
from concourse.bass_utils import run_bass_kernel_spmd

N, C, D = 8192, 10000, 512
N_CORES = 8
NS = N // N_CORES       # 1024 rows per core
P = 128                 # partitions
M_TILES = NS // P       # 8 row tiles per core
K2 = D // (2 * P)       # 2 DoubleRow contraction tiles (256 rows each)
CW = 512                # matmul free-dim tile (one PSUM bank of fp32)
GW = 2048               # PSUM megatile width: 4 banks, one ACTIVATE each
G_TILES = (C + GW - 1) // GW  # 5 (4 x 2048 + 1808)
SCALE = 5.0
EPS = 1e-12
FP8_SCALE = 512.0       # 2^9: keeps |values| <= ~120 within e4m3 normal range
FP8 = ml_dtypes.float8_e4m3
N_ACT_ACCUM = 5         # megatiles row-summed via ACT accum_out (rest on DVE)

_compiled = None
LAST_RESULTS = None


def _build():
    nc = bacc.Bacc(
        "TRN2",
        target_bir_lowering=False,
        debug=False,
        enable_asserts=False,
        num_devices=N_CORES,
    )
    # strip-major: per strip each partition's row is 8 KB contiguous in DRAM
    ct_d = nc.dram_tensor(
        "ct", [G_TILES, P, K2, 2, GW], mybir.dt.float8e4, kind="ExternalInput"
    ).ap()
    ft_d = nc.dram_tensor(
        "ft", [P, K2, 2, NS], mybir.dt.float8e4, kind="ExternalInput"
    ).ap()
    ab_d = nc.dram_tensor("ab", [P, M_TILES], mybir.dt.float32, kind="ExternalInput").ap()
    # per-(g,m) partial row-sums (col = g*M + m); host does the final combine
    rs_d = nc.dram_tensor(
        "rs", [P, G_TILES * M_TILES], mybir.dt.float32, kind="ExternalOutput"
    ).ap()

    with tile.TileContext(nc) as tc:
        with (
            tc.tile_pool(name="cpool", bufs=1) as cpool,
            tc.tile_pool(name="fpool", bufs=1) as fpool,
            tc.tile_pool(name="spool", bufs=1) as spool,
            tc.tile_pool(name="epool", bufs=3) as epool,
            tc.tile_pool(name="partpool", bufs=1) as partpool,
            tc.tile_pool(name="ppool", bufs=2, space="PSUM") as ppool,
        ):
            # spread the input fill across engine DMA queues (PE/DVE streams
            # are idle early, so their issue slots are free and the HWDGE
            # rings run in parallel)
            ct0a = cpool.tile([P, K2, 2, GW // 2], mybir.dt.float8e4, tag="ct0a")
            nc.scalar.dma_start(out=ct0a[:], in_=ct_d[0][:, :, :, 0 : GW // 2])

            ft_sb = fpool.tile([P, K2, 2, NS], mybir.dt.float8e4, tag="ft")
            nc.sync.dma_start(out=ft_sb[:], in_=ft_d)

            bias_sb = spool.tile([P, M_TILES], mybir.dt.float32, tag="bias")
            nc.sync.dma_start(out=bias_sb[:], in_=ab_d)

            ct0b = cpool.tile([P, K2, 2, GW // 2], mybir.dt.float8e4, tag="ct0b")
            nc.scalar.dma_start(out=ct0b[:], in_=ct_d[0][:, :, :, GW // 2 : GW])

            # remaining strips: one DMA per strip (both DoubleRow k-tiles)
            ct_sb = [None]
            for g in range(1, G_TILES):
                gw = min(GW, C - g * GW)
                t = cpool.tile(
                    [P, K2, 2, GW], mybir.dt.float8e4, tag=f"ct{g}", name=f"ct{g}"
                )
                eng = nc.gpsimd if g % 2 else nc.sync
                eng.dma_start(out=t[:, :, :, :gw], in_=ct_d[g][:, :, :, :gw])
                ct_sb.append(t)

            # partial row-sum accumulators: ACT's accum_out for the first few
            # megatiles, DVE cache-reduce for the rest (balances both engines)
            parts_act = partpool.tile([P, N_ACT_ACCUM], mybir.dt.float32, tag="pa")
            parts_dve = partpool.tile(
                [P, G_TILES * M_TILES - N_ACT_ACCUM], mybir.dt.float32, tag="pd"
            )

            # strip-outer / row-tile-inner: PE is dense as soon as strip 0 lands
            for g in range(G_TILES):
                gw = min(GW, C - g * GW)
                n_sl = (gw + CW - 1) // CW
                for m in range(M_TILES):
                    tile_idx = g * M_TILES + m
                    # ACT-accum handles the LAST megatiles so the kernel tail
                    # doesn't wait on a trailing DVE reduce
                    on_act = tile_idx >= G_TILES * M_TILES - N_ACT_ACCUM
                    ps = ppool.tile([P, GW], mybir.dt.float32, tag="ps")
                    for k in range(K2):
                        for j in range(n_sl):
                            w = min(CW, gw - j * CW)
                            if g == 0:
                                rhs = (ct0a if j < 2 else ct0b)[
                                    :, k, :, (j % 2) * CW : (j % 2) * CW + w
                                ]
                            else:
                                rhs = ct_sb[g][:, k, :, j * CW : j * CW + w]
                            nc.tensor.matmul(
                                ps[:, j * CW : j * CW + w],
                                ft_sb[:, k, :, m * P : (m + 1) * P],
                                rhs,
                                start=(k == 0),
                                stop=(k == K2 - 1),
                                perf_mode=mybir.MatmulPerfMode.DoubleRow,
                                skip_group_check=True,
                            )
                    et = epool.tile([P, GW], mybir.dt.bfloat16, tag="exp")
                    nc.scalar.activation(
                        et[:, :gw],
                        ps[:, :gw],
                        mybir.ActivationFunctionType.Exp,
                        bias=bias_sb[:, m : m + 1],
                        scale=2.0 * SCALE / (FP8_SCALE * FP8_SCALE),
                        accum_out=(
                            parts_act[
                                :,
                                tile_idx - (G_TILES * M_TILES - N_ACT_ACCUM) : tile_idx
                                - (G_TILES * M_TILES - N_ACT_ACCUM)
                                + 1,
                            ]
                            if on_act
                            else None
                        ),
                    )
                    if not on_act:
                        # row-sum of the bf16 exp tile on the vector engine
                        nc.vector.tensor_scalar(
                            et[:, :gw],
                            et[:, :gw],
                            1.0,
                            None,
                            op0=mybir.AluOpType.mult,
                            op1=mybir.AluOpType.add,
                            accum_out=parts_dve[:, tile_idx : tile_idx + 1],
                        )
            n_dve = G_TILES * M_TILES - N_ACT_ACCUM
            nc.sync.dma_start(out=rs_d[:, 0:n_dve], in_=parts_dve[:])
            nc.sync.dma_start(out=rs_d[:, n_dve:], in_=parts_act[:])

    nc.compile()
    return nc


def _get_compiled():
    global _compiled
    if _compiled is None:
        _compiled = _build()
    return _compiled


def _l2n(x):
    n = np.sqrt(np.einsum("nd,nd->n", x, x, dtype=np.float32), dtype=np.float32)
    xh = x / np.maximum(n, np.float32(EPS))[:, None]
    sq = np.einsum("nd,nd->n", xh, xh, dtype=np.float32)
    return xh.astype(np.float32), sq.astype(np.float32)


def _pack_dr(xt):
    """[D, W] fp32 (pre-scaled) -> DoubleRow fp8 [P, K2, 2, W]:
    row d = k*256 + i*128 + p  ->  out[p, k, i]."""
    d, w = xt.shape
    return np.ascontiguousarray(
        xt.reshape(K2, 2, P, w).transpose(2, 0, 1, 3)
    ).astype(FP8)


def _pack_ct(xt):
    """[D, C] fp32 (pre-scaled) -> strip-major DoubleRow fp8
    [G_TILES, P, K2, 2, GW] (last strip zero-padded)."""
    out = np.zeros((G_TILES, P, K2, 2, GW), dtype=FP8)
    for g in range(G_TILES):
        gw = min(GW, C - g * GW)
        out[g, :, :, :, :gw] = _pack_dr(xt[:, g * GW : g * GW + gw])
    return out


def _combine_rs(rs):
    """[P, G*M] per-core output (col = tile_idx = g*M + m) -> per-row sums
    [NS] (n = m*128 + p)."""
    out = rs.astype(np.float64).reshape(P, G_TILES, M_TILES).sum(axis=1)
    return out.T.reshape(NS)


def kernel(features, labels, centers, bias):
    features = np.asarray(features, dtype=np.float32)
    centers = np.asarray(centers, dtype=np.float32)
    bias = np.asarray(bias, dtype=np.float32)
    labels_i = np.asarray(labels).astype(np.int64)

    fh, f2 = _l2n(features)          # [N, D], [N]
    ch, c2 = _l2n(centers)           # [C, D], [C]

    ct8 = _pack_ct(ch.T * np.float32(FP8_SCALE))            # [G, P, K2, 2, GW]
    abias_full = (-SCALE * (f2 + np.float32(1.0))).astype(np.float32)

    in_maps = []
    for i in range(N_CORES):
        sl = slice(i * NS, (i + 1) * NS)
        ft8 = _pack_dr(fh[sl].T * np.float32(FP8_SCALE))    # [P, K2, 2, NS]
        ab = np.ascontiguousarray(
            abias_full[sl].reshape(M_TILES, P).T
        )  # [P, M_TILES], n = m*128 + p
        in_maps.append({"ct": ct8, "ft": ft8, "ab": ab})

    nc = _get_compiled()
    global LAST_RESULTS
    LAST_RESULTS = run_bass_kernel_spmd(nc, in_maps, core_ids=list(range(N_CORES)))

    rowsum = np.concatenate(
        [_combine_rs(LAST_RESULTS.results[i]["rs"]) for i in range(N_CORES)]
    ).astype(np.float64)

    # residual correction for the |c_k|^2 ~= 1 fold (mean of exp(-5*(c2-1)))
    wmean = np.exp(-SCALE * (c2.astype(np.float64) - 1.0)).mean()
    rowsum *= wmean

    # exact per-row label terms (fp32 inputs, fp64 math)
    cl = ch[labels_i]                                        # [N, D]
    dot = np.einsum("nd,nd->n", fh.astype(np.float64), cl.astype(np.float64))
    dis_l = -SCALE * (f2.astype(np.float64) + c2[labels_i].astype(np.float64) - 2.0 * dot)
    pos = dis_l + bias[labels_i, 0].astype(np.float64)

    num = np.exp(pos)
    den = rowsum - np.exp(dis_l) + num
    logits = np.log(den) - pos
    variance = np.var(pos, ddof=1)
    loss = logits.mean() + variance
    return (np.float32(loss), np.float32(variance))


# revision 39
# speedup vs baseline: 1.0240x; 1.0240x over previous
"""Trainium2 Bass kernel for nn_CenterLossNet (center-loss softmax over classes).

Math (reference):
    f = l2_normalize(features); c = l2_normalize(centers)
    dis[n,k]  = -5 * (|f_n|^2 + |c_k|^2 - 2 f_n.c_k)        # [N, C]
    pos[n]    = dis[n, labels[n]] + bias[labels[n]]
    den[n]    = sum_k exp(dis[n,k]) - exp(dis[n,l_n]) + exp(pos[n])
    loss      = mean(log(den) - pos) + var(pos, ddof=1);  returns (loss, var)

Device does the heavy part: S = f_hat @ c_hat.T (8192x10000x512 matmul) fused
with exp(10*S + bias_n) on the scalar engine. The matmul runs in fp8e4m3
DoubleRow perf mode (operands pre-scaled by 2^9, two k-rows packed per PE
cell); the exp-sum averages the fp8 rounding noise down to ~1e-5 relative on
the loss. Row-sums of the bf16 exp tiles run on the vector engine
(tensor_scalar 4x mode with accum_out). Everything O(N) or O(C) runs on host
in fp64, so pos/variance use exact fp32 inputs.

Sharding: data-parallel over batch N across 8 cores; centers replicated.
For the row-sum the per-class |c_k|^2 term is folded as exactly 1.0 (the
normalized squared norms differ from 1 by ~1e-6, and the host applies the
mean residual correction), while pos[n] uses the exact fp32 per-label norms.
"""

import numpy as np
import ml_dtypes

import concourse.bacc as bacc
import concourse.mybir as mybir
import concourse.tile as tile
from concourse.bass_utils import run_bass_kernel_spmd

N, C, D = 8192, 10000, 512
N_CORES = 8
NS = N // N_CORES       # 1024 rows per core
P = 128                 # partitions
M_TILES = NS // P       # 8 row tiles per core
K2 = D // (2 * P)       # 2 DoubleRow contraction tiles (256 rows each)
CW = 512                # matmul free-dim tile (one PSUM bank of fp32)
GW = 2048               # PSUM megatile width: 4 banks, one ACTIVATE each
G_TILES = (C + GW - 1) // GW  # 5 (4 x 2048 + 1808)
SCALE = 5.0
EPS = 1e-12
FP8_SCALE = 512.0       # 2^9: keeps |values| <= ~120 within e4m3 normal range
FP8 = ml_dtypes.float8_e4m3
N_ACT_ACCUM = 5         # megatiles row-summed via ACT accum_out (rest on DVE)

_compiled = None
LAST_RESULTS = None


def _build():
    nc = bacc.Bacc(
        "TRN2",
        target_bir_lowering=False,
        debug=False,
        enable_asserts=False,
        num_devices=N_CORES,
    )
    # strip-major: per strip each partition's row is 8 KB contiguous in DRAM
    ct_d = nc.dram_tensor(
        "ct", [G_TILES, P, K2, 2, GW], mybir.dt.float8e4, kind="ExternalInput"
    ).ap()
    ft_d = nc.dram_tensor(
        "ft", [P, K2, 2, NS], mybir.dt.float8e4, kind="ExternalInput"
    ).ap()
    ab_d = nc.dram_tensor("ab", [P, M_TILES], mybir.dt.float32, kind="ExternalInput").ap()
    # per-(g,m) partial row-sums (col = g*M + m); host does the final combine
    rs_d = nc.dram_tensor(
        "rs", [P, G_TILES * M_TILES], mybir.dt.float32, kind="ExternalOutput"
    ).ap()

    with tile.TileContext(nc) as tc:
        with (
            tc.tile_pool(name="cpool", bufs=1) as cpool,
            tc.tile_pool(name="fpool", bufs=1) as fpool,
            tc.tile_pool(name="spool", bufs=1) as spool,
            tc.tile_pool(name="epool", bufs=3) as epool,
            tc.tile_pool(name="partpool", bufs=1) as partpool,
            tc.tile_pool(name="ppool", bufs=2, space="PSUM") as ppool,
        ):
            # spread the input fill across engine DMA queues (PE/DVE streams
            # are idle early, so their issue slots are free and the HWDGE
            # rings run in parallel)
            # critical prefix on the fast sync ring, in first-use order;
            # ft split by k-tile so the very first matmul starts sooner
            ft_sb = []
            for k in range(K2):
                t = fpool.tile([P, 2, NS], mybir.dt.float8e4, tag=f"ft{k}", name=f"ft{k}")
                nc.sync.dma_start(out=t[:], in_=ft_d[:, k])
                ft_sb.append(t)

            ct0a = cpool.tile([P, K2, 2, GW // 2], mybir.dt.float8e4, tag="ct0a")
            nc.sync.dma_start(out=ct0a[:], in_=ct_d[0][:, :, :, 0 : GW // 2])

            bias_sb = spool.tile([P, M_TILES], mybir.dt.float32, tag="bias")
            nc.sync.dma_start(out=bias_sb[:], in_=ab_d)

            ct0b = cpool.tile([P, K2, 2, GW // 2], mybir.dt.float8e4, tag="ct0b")
            nc.sync.dma_start(out=ct0b[:], in_=ct_d[0][:, :, :, GW // 2 : GW])

            # remaining strips: one DMA per strip; late strips go to the
            # (slower) scalar ring, which has plenty of slack
            ct_sb = [None]
            for g in range(1, G_TILES):
                gw = min(GW, C - g * GW)
                t = cpool.tile(
                    [P, K2, 2, GW], mybir.dt.float8e4, tag=f"ct{g}", name=f"ct{g}"
                )
                eng = nc.sync if g == 1 else nc.scalar
                eng.dma_start(out=t[:, :, :, :gw], in_=ct_d[g][:, :, :, :gw])
                ct_sb.append(t)

            # partial row-sum accumulators: ACT's accum_out for the first few
            # megatiles, DVE cache-reduce for the rest (balances both engines)
            parts_act = partpool.tile([P, N_ACT_ACCUM], mybir.dt.float32, tag="pa")
            parts_dve = partpool.tile(
                [P, G_TILES * M_TILES - N_ACT_ACCUM], mybir.dt.float32, tag="pd"
            )

            # strip-outer / row-tile-inner: PE is dense as soon as strip 0 lands
            for g in range(G_TILES):
                gw = min(GW, C - g * GW)
                n_sl = (gw + CW - 1) // CW
                for m in range(M_TILES):
                    tile_idx = g * M_TILES + m
                    # ACT-accum handles the LAST megatiles so the kernel tail
                    # doesn't wait on a trailing DVE reduce
                    on_act = tile_idx >= G_TILES * M_TILES - N_ACT_ACCUM
                    ps = ppool.tile([P, GW], mybir.dt.float32, tag="ps")
                    for k in range(K2):
                        for j in range(n_sl):
                            w = min(CW, gw - j * CW)
                            if g == 0:
                                rhs = (ct0a if j < 2 else ct0b)[
                                    :, k, :, (j % 2) * CW : (j % 2) * CW + w
                                ]
                            else:
                                rhs = ct_sb[g][:, k, :, j * CW : j * CW + w]
                            nc.tensor.matmul(
                                ps[:, j * CW : j * CW + w],
                                ft_sb[k][:, :, m * P : (m + 1) * P],
                                rhs,
                                start=(k == 0),
                                stop=(k == K2 - 1),
                                perf_mode=mybir.MatmulPerfMode.DoubleRow,
                                skip_group_check=True,
                            )
                    et = epool.tile([P, GW], mybir.dt.bfloat16, tag="exp")
                    nc.scalar.activation(
                        et[:, :gw],
                        ps[:, :gw],
                        mybir.ActivationFunctionType.Exp,
                        bias=bias_sb[:, m : m + 1],
                        scale=2.0 * SCALE / (FP8_SCALE * FP8_SCALE),
                        accum_out=(
                            parts_act[
                                :,
                                tile_idx - (G_TILES * M_TILES - N_ACT_ACCUM) : tile_idx
                                - (G_TILES * M_TILES - N_ACT_ACCUM)
                                + 1,
                            ]
                            if on_act
                            else None
                        ),
                    )
                    if not on_act:
                        # row-sum of the bf16 exp tile on the vector engine
                        nc.vector.tensor_scalar(
                            et[:, :gw],
                            et[:, :gw],
                            1.0,
                            None,
                            op0=mybir.AluOpType.mult,
                            op1=mybir.AluOpType.add,
                            accum_out=parts_dve[:, tile_idx : tile_idx + 1],
                        )
            n_dve = G_TILES * M_TILES - N_ACT_ACCUM
            nc.sync.dma_start(out=rs_d[:, 0:n_dve], in_=parts_dve[:])
            nc.sync.dma_start(out=rs_d[:, n_dve:], in_=parts_act[:])

    nc.compile()
    return nc


def _get_compiled():
    global _compiled
    if _compiled is None:
        _compiled = _build()
    return _compiled


def _l2n(x):
    n = np.sqrt(np.einsum("nd,nd->n", x, x, dtype=np.float32), dtype=np.float32)
    xh = x / np.maximum(n, np.float32(EPS))[:, None]
    sq = np.einsum("nd,nd->n", xh, xh, dtype=np.float32)
    return xh.astype(np.float32), sq.astype(np.float32)


def _pack_dr(xt):
    """[D, W] fp32 (pre-scaled) -> DoubleRow fp8 [P, K2, 2, W]:
    row d = k*256 + i*128 + p  ->  out[p, k, i]."""
    d, w = xt.shape
    return np.ascontiguousarray(
        xt.reshape(K2, 2, P, w).transpose(2, 0, 1, 3)
    ).astype(FP8)


def _pack_ct(xt):
    """[D, C] fp32 (pre-scaled) -> strip-major DoubleRow fp8
    [G_TILES, P, K2, 2, GW] (last strip zero-padded)."""
    out = np.zeros((G_TILES, P, K2, 2, GW), dtype=FP8)
    for g in range(G_TILES):
        gw = min(GW, C - g * GW)
        out[g, :, :, :, :gw] = _pack_dr(xt[:, g * GW : g * GW + gw])
    return out


def _combine_rs(rs):
    """[P, G*M] per-core output (col = tile_idx = g*M + m) -> per-row sums
    [NS] (n = m*128 + p)."""
    out = rs.astype(np.float64).reshape(P, G_TILES, M_TILES).sum(axis=1)
    return out.T.reshape(NS)


def kernel(features, labels, centers, bias):
    features = np.asarray(features, dtype=np.float32)
    centers = np.asarray(centers, dtype=np.float32)
    bias = np.asarray(bias, dtype=np.float32)
    labels_i = np.asarray(labels).astype(np.int64)

    fh, f2 = _l2n(features)          # [N, D], [N]
    ch, c2 = _l2n(centers)           # [C, D], [C]

    ct8 = _pack_ct(ch.T * np.float32(FP8_SCALE))            # [G, P, K2, 2, GW]
    abias_full = (-SCALE * (f2 + np.float32(1.0))).astype(np.float32)

    in_maps = []
    for i in range(N_CORES):
        sl = slice(i * NS, (i + 1) * NS)
        ft8 = _pack_dr(fh[sl].T * np.float32(FP8_SCALE))    # [P, K2, 2, NS]
        ab = np.ascontiguousarray(
            abias_full[sl].reshape(M_TILES, P).T
        )  # [P, M_TILES], n = m*128 + p
        in_maps.append({"ct": ct8, "ft": ft8, "ab": ab})

    nc = _get_compiled()
    global LAST_RESULTS
    LAST_RESULTS = run_bass_kernel_spmd(nc, in_maps, core_ids=list(range(N_CORES)))

    rowsum = np.concatenate(
        [_combine_rs(LAST_RESULTS.results[i]["rs"]) for i in range(N_CORES)]
    ).astype(np.float64)

    # residual correction for the |c_k|^2 ~= 1 fold (mean of exp(-5*(c2-1)))
    wmean = np.exp(-SCALE * (c2.astype(np.float64) - 1.0)).mean()
    rowsum *= wmean

    # exact per-row label terms (fp32 inputs, fp64 math)
    cl = ch[labels_i]                                        # [N, D]
    dot = np.einsum("nd,nd->n", fh.astype(np.float64), cl.astype(np.float64))
    dis_l = -SCALE * (f2.astype(np.float64) + c2[labels_i].astype(np.float64) - 2.0 * dot)
    pos = dis_l + bias[labels_i, 0].astype(np.float64)

    num = np.exp(pos)
    den = rowsum - np.exp(dis_l) + num
    logits = np.log(den) - pos
    variance = np.var(pos, ddof=1)
    loss = logits.mean() + variance
    return (np.float32(loss), np.float32(variance))


# revision 45
# speedup vs baseline: 1.0529x; 1.0282x over previous
"""Trainium2 Bass kernel for nn_CenterLossNet (center-loss softmax over classes).

Math (reference):
    f = l2_normalize(features); c = l2_normalize(centers)
    dis[n,k]  = -5 * (|f_n|^2 + |c_k|^2 - 2 f_n.c_k)        # [N, C]
    pos[n]    = dis[n, labels[n]] + bias[labels[n]]
    den[n]    = sum_k exp(dis[n,k]) - exp(dis[n,l_n]) + exp(pos[n])
    loss      = mean(log(den) - pos) + var(pos, ddof=1);  returns (loss, var)

Device does the heavy part: S = f_hat @ c_hat.T (8192x10000x512 matmul) fused
with exp(10*S + bias_n) on the scalar engine. The matmul runs in fp8e4m3
DoubleRow perf mode (operands pre-scaled by 2^9, two k-rows packed per PE
cell); the exp-sum averages the fp8 rounding noise down to ~1e-5 relative on
the loss. Row-sums of the bf16 exp tiles run on the vector engine
(tensor_scalar 4x mode with accum_out). Everything O(N) or O(C) runs on host
in fp64, so pos/variance use exact fp32 inputs.

Sharding: data-parallel over batch N across 8 cores; centers replicated.
For the row-sum the per-class |c_k|^2 term is folded as exactly 1.0 (the
normalized squared norms differ from 1 by ~1e-6, and the host applies the
mean residual correction), while pos[n] uses the exact fp32 per-label norms.
"""

import numpy as np
import ml_dtypes

import concourse.bacc as bacc
import concourse.mybir as mybir
import concourse.tile as tile
from concourse.bass_utils import run_bass_kernel_spmd

N, C, D = 8192, 10000, 512
N_CORES = 8
NS = N // N_CORES       # 1024 rows per core
P = 128                 # partitions
M_TILES = NS // P       # 8 row tiles per core
K2 = D // (2 * P)       # 2 DoubleRow contraction tiles (256 rows each)
CW = 512                # matmul free-dim tile (one PSUM bank of fp32)
GW = 2048               # PSUM megatile width: 4 banks, one ACTIVATE each
G_TILES = (C + GW - 1) // GW  # 5 (4 x 2048 + 1808)
SCALE = 5.0
EPS = 1e-12
FP8_SCALE = 512.0       # 2^9: keeps |values| <= ~120 within e4m3 normal range
FP8 = ml_dtypes.float8_e4m3
N_ACT_ACCUM = 5         # megatiles row-summed via ACT accum_out (rest on DVE)

_compiled = None
LAST_RESULTS = None


def _build():
    nc = bacc.Bacc(
        "TRN2",
        target_bir_lowering=False,
        debug=False,
        enable_asserts=False,
        num_devices=N_CORES,
    )
    # strip-major: per strip each partition's row is 8 KB contiguous in DRAM;
    # strip 0 is stored as two contiguous half-strips for a fast first fill
    ct0_d = nc.dram_tensor(
        "ct0", [2, P, K2, 2, GW // 2], mybir.dt.float8e4, kind="ExternalInput"
    ).ap()
    ct_d = nc.dram_tensor(
        "ct", [G_TILES - 1, P, K2, 2, GW], mybir.dt.float8e4, kind="ExternalInput"
    ).ap()
    ft_d = nc.dram_tensor(
        "ft", [P, K2, 2, NS], mybir.dt.float8e4, kind="ExternalInput"
    ).ap()
    ab_d = nc.dram_tensor("ab", [P, M_TILES], mybir.dt.float32, kind="ExternalInput").ap()
    # per-(g,m) partial row-sums (col = g*M + m); host does the final combine
    rs_d = nc.dram_tensor(
        "rs", [P, G_TILES * M_TILES], mybir.dt.float32, kind="ExternalOutput"
    ).ap()

    with tile.TileContext(nc) as tc:
        with (
            tc.tile_pool(name="cpool", bufs=1) as cpool,
            tc.tile_pool(name="fpool", bufs=1) as fpool,
            tc.tile_pool(name="spool", bufs=1) as spool,
            tc.tile_pool(name="epool", bufs=3) as epool,
            tc.tile_pool(name="partpool", bufs=1) as partpool,
            tc.tile_pool(name="ppool", bufs=2, space="PSUM") as ppool,
        ):
            # spread the input fill across engine DMA queues (PE/DVE streams
            # are idle early, so their issue slots are free and the HWDGE
            # rings run in parallel)
            # warm the PE clock (HAM) with throwaway DoubleRow matmuls on a
            # zeroed tile while the first input DMAs are still in flight
            z8 = spool.tile([P, 2, CW], mybir.dt.float8e4, tag="z8")
            nc.gpsimd.memset(z8[:], 0.0)
            wps = ppool.tile([P, CW], mybir.dt.float32, tag="ps", name="wps")
            for _ in range(20):
                nc.tensor.matmul(
                    wps[:],
                    z8[:, :, 0:P],
                    z8[:],
                    start=True,
                    stop=True,
                    perf_mode=mybir.MatmulPerfMode.DoubleRow,
                    skip_group_check=True,
                )

            # critical prefix on the fast sync ring, in first-use order
            ct0a = cpool.tile([P, K2, 2, GW // 2], mybir.dt.float8e4, tag="ct0a")
            nc.sync.dma_start(out=ct0a[:], in_=ct0_d[0])

            ft_sb = fpool.tile([P, K2, 2, NS], mybir.dt.float8e4, tag="ft")
            nc.sync.dma_start(out=ft_sb[:], in_=ft_d)

            bias_sb = spool.tile([P, M_TILES], mybir.dt.float32, tag="bias")
            nc.sync.dma_start(out=bias_sb[:], in_=ab_d)

            ct0b = cpool.tile([P, K2, 2, GW // 2], mybir.dt.float8e4, tag="ct0b")
            nc.sync.dma_start(out=ct0b[:], in_=ct0_d[1])

            # remaining strips: one DMA per strip; the last two go to the
            # (slower) scalar ring, which has plenty of slack
            ct_sb = [None]
            for g in range(1, G_TILES):
                gw = min(GW, C - g * GW)
                t = cpool.tile(
                    [P, K2, 2, GW], mybir.dt.float8e4, tag=f"ct{g}", name=f"ct{g}"
                )
                eng = nc.sync if g <= 2 else nc.scalar
                eng.dma_start(out=t[:, :, :, :gw], in_=ct_d[g - 1][:, :, :, :gw])
                ct_sb.append(t)

            # partial row-sum accumulators: ACT's accum_out for the first few
            # megatiles, DVE cache-reduce for the rest (balances both engines)
            parts_act = partpool.tile([P, N_ACT_ACCUM], mybir.dt.float32, tag="pa")
            parts_dve = partpool.tile(
                [P, G_TILES * M_TILES - N_ACT_ACCUM], mybir.dt.float32, tag="pd"
            )

            # strip-outer / row-tile-inner: PE is dense as soon as strip 0 lands
            for g in range(G_TILES):
                gw = min(GW, C - g * GW)
                n_sl = (gw + CW - 1) // CW
                for m in range(M_TILES):
                    tile_idx = g * M_TILES + m
                    # ACT-accum handles the LAST megatiles so the kernel tail
                    # doesn't wait on a trailing DVE reduce
                    on_act = tile_idx >= G_TILES * M_TILES - N_ACT_ACCUM
                    ps = ppool.tile([P, GW], mybir.dt.float32, tag="ps")
                    for k in range(K2):
                        for j in range(n_sl):
                            w = min(CW, gw - j * CW)
                            if g == 0:
                                rhs = (ct0a if j < 2 else ct0b)[
                                    :, k, :, (j % 2) * CW : (j % 2) * CW + w
                                ]
                            else:
                                rhs = ct_sb[g][:, k, :, j * CW : j * CW + w]
                            nc.tensor.matmul(
                                ps[:, j * CW : j * CW + w],
                                ft_sb[:, k, :, m * P : (m + 1) * P],
                                rhs,
                                start=(k == 0),
                                stop=(k == K2 - 1),
                                perf_mode=mybir.MatmulPerfMode.DoubleRow,
                                skip_group_check=True,
                            )
                    et = epool.tile([P, GW], mybir.dt.bfloat16, tag="exp")
                    nc.scalar.activation(
                        et[:, :gw],
                        ps[:, :gw],
                        mybir.ActivationFunctionType.Exp,
                        bias=bias_sb[:, m : m + 1],
                        scale=2.0 * SCALE / (FP8_SCALE * FP8_SCALE),
                        accum_out=(
                            parts_act[
                                :,
                                tile_idx - (G_TILES * M_TILES - N_ACT_ACCUM) : tile_idx
                                - (G_TILES * M_TILES - N_ACT_ACCUM)
                                + 1,
                            ]
                            if on_act
                            else None
                        ),
                    )
                    if not on_act:
                        # row-sum of the bf16 exp tile on the vector engine
                        nc.vector.tensor_scalar(
                            et[:, :gw],
                            et[:, :gw],
                            1.0,
                            None,
                            op0=mybir.AluOpType.mult,
                            op1=mybir.AluOpType.add,
                            accum_out=parts_dve[:, tile_idx : tile_idx + 1],
                        )
            n_dve = G_TILES * M_TILES - N_ACT_ACCUM
            nc.sync.dma_start(out=rs_d[:, 0:n_dve], in_=parts_dve[:])
            nc.sync.dma_start(out=rs_d[:, n_dve:], in_=parts_act[:])

    nc.compile()
    return nc


def _get_compiled():
    global _compiled
    if _compiled is None:
        _compiled = _build()
    return _compiled


def _l2n(x):
    n = np.sqrt(np.einsum("nd,nd->n", x, x, dtype=np.float32), dtype=np.float32)
    xh = x / np.maximum(n, np.float32(EPS))[:, None]
    sq = np.einsum("nd,nd->n", xh, xh, dtype=np.float32)
    return xh.astype(np.float32), sq.astype(np.float32)


def _pack_dr(xt):
    """[D, W] fp32 (pre-scaled) -> DoubleRow fp8 [P, K2, 2, W]:
    row d = k*256 + i*128 + p  ->  out[p, k, i]."""
    d, w = xt.shape
    return np.ascontiguousarray(
        xt.reshape(K2, 2, P, w).transpose(2, 0, 1, 3)
    ).astype(FP8)


def _pack_ct(xt):
    """[D, C] fp32 (pre-scaled) -> (strip-0 halves [2, P, K2, 2, GW/2],
    strips 1.. [G-1, P, K2, 2, GW], last zero-padded)."""
    ct0 = np.stack(
        [_pack_dr(xt[:, 0 : GW // 2]), _pack_dr(xt[:, GW // 2 : GW])]
    )
    ctr = np.zeros((G_TILES - 1, P, K2, 2, GW), dtype=FP8)
    for g in range(1, G_TILES):
        gw = min(GW, C - g * GW)
        ctr[g - 1, :, :, :, :gw] = _pack_dr(xt[:, g * GW : g * GW + gw])
    return ct0, ctr


def _combine_rs(rs):
    """[P, G*M] per-core output (col = tile_idx = g*M + m) -> per-row sums
    [NS] (n = m*128 + p)."""
    out = rs.astype(np.float64).reshape(P, G_TILES, M_TILES).sum(axis=1)
    return out.T.reshape(NS)


def kernel(features, labels, centers, bias):
    features = np.asarray(features, dtype=np.float32)
    centers = np.asarray(centers, dtype=np.float32)
    bias = np.asarray(bias, dtype=np.float32)
    labels_i = np.asarray(labels).astype(np.int64)

    fh, f2 = _l2n(features)          # [N, D], [N]
    ch, c2 = _l2n(centers)           # [C, D], [C]

    ct0_8, ct8 = _pack_ct(ch.T * np.float32(FP8_SCALE))
    abias_full = (-SCALE * (f2 + np.float32(1.0))).astype(np.float32)

    in_maps = []
    for i in range(N_CORES):
        sl = slice(i * NS, (i + 1) * NS)
        ft8 = _pack_dr(fh[sl].T * np.float32(FP8_SCALE))    # [P, K2, 2, NS]
        ab = np.ascontiguousarray(
            abias_full[sl].reshape(M_TILES, P).T
        )  # [P, M_TILES], n = m*128 + p
        in_maps.append({"ct0": ct0_8, "ct": ct8, "ft": ft8, "ab": ab})

    nc = _get_compiled()
    global LAST_RESULTS
    LAST_RESULTS = run_bass_kernel_spmd(nc, in_maps, core_ids=list(range(N_CORES)))

    rowsum = np.concatenate(
        [_combine_rs(LAST_RESULTS.results[i]["rs"]) for i in range(N_CORES)]
    ).astype(np.float64)

    # residual correction for the |c_k|^2 ~= 1 fold (mean of exp(-5*(c2-1)))
    wmean = np.exp(-SCALE * (c2.astype(np.float64) - 1.0)).mean()
    rowsum *= wmean

    # exact per-row label terms (fp32 inputs, fp64 math)
    cl = ch[labels_i]                                        # [N, D]
    dot = np.einsum("nd,nd->n", fh.astype(np.float64), cl.astype(np.float64))
    dis_l = -SCALE * (f2.astype(np.float64) + c2[labels_i].astype(np.float64) - 2.0 * dot)
    pos = dis_l + bias[labels_i, 0].astype(np.float64)

    num = np.exp(pos)
    den = rowsum - np.exp(dis_l) + num
    logits = np.log(den) - pos
    variance = np.var(pos, ddof=1)
    loss = logits.mean() + variance
    return (np.float32(loss), np.float32(variance))
